# revision 19
# baseline (speedup 1.0000x reference)
"""GQA MultiHeadAttention (RoPE, causal) Bass/Tile kernel for 8 Trainium2 cores.

Problem: x[2,2048,2048] @ Wq/Wk/Wv -> RoPE -> causal GQA attention -> @ Wo.
D=2048, H=16 heads, G=4 KV groups, HD=128, B=2, S=2048.

Sharding (SPMD, one program, per-core data):
  core c -> batch b=c//4, KV-group g=c%4 (heads 4g..4g+3).
  Each core: QKV projection for its group from x[b]^T, RoPE, 4 heads of
  attention, and a row-shard of the output projection (Wo rows for its
  heads) producing a partial [2048,2048] output. Host sums the 4 partials
  per batch.

On-chip layouts are all "transposed" (feature dim on partitions):
  QT/KT/VT [hd, s]; scores computed as scoresT [k, q]; softmax denominator
  via ones-vector matmul (sum over partition dim); ctxT [hd, q];
  out-projection uses ctxT slices as stationary to produce natural [s, d].
fp32r (single-pass PE fp32) everywhere on the matmul path.
"""

import sys

if "/opt/trn_rl_repo" not in sys.path:
    sys.path.insert(0, "/opt/trn_rl_repo")

from contextlib import ExitStack

import numpy as np

import concourse.bass as bass
import concourse.tile as tile
from concourse import bacc, mybir
from concourse.bass_utils import run_bass_kernel_spmd
from concourse.masks import make_identity

F32 = mybir.dt.float32
F32R = mybir.dt.float32r
BF16 = mybir.dt.bfloat16
AF = mybir.ActivationFunctionType

B, S, D = 2, 2048, 2048
H, G, HD = 16, 4, 128
HPG = H // G          # heads per group = 4
GD = HPG * HD         # group width = 512
P = 128
NCHUNK = 512          # matmul moving free dim
SC = S // NCHUNK      # 4 s-chunks
DT = D // P           # 16 d-tiles
ST = S // P           # 16 s-tiles
SCALE = 1.0 / float(np.sqrt(HD))

_CACHE = {}


def _build():
    nc = bacc.Bacc("TRN2", target_bir_lowering=False, debug=False, num_devices=8)

    # ---- DRAM I/O (per-core shards) ----
    xT = nc.dram_tensor("xT", [D, S], F32R, kind="ExternalInput").ap()
    wq = nc.dram_tensor("wq", [D, GD], F32R, kind="ExternalInput").ap()
    wk = nc.dram_tensor("wk", [D, HD], F32R, kind="ExternalInput").ap()
    wv = nc.dram_tensor("wv", [D, HD], F32R, kind="ExternalInput").ap()
    wo = nc.dram_tensor("wo", [GD, D], F32R, kind="ExternalInput").ap()
    cosT = nc.dram_tensor("cosT", [HD, S], F32, kind="ExternalInput").ap()
    sinT = nc.dram_tensor("sinT", [HD, S], F32, kind="ExternalInput").ap()
    prot = nc.dram_tensor("prot", [HD, HD], F32R, kind="ExternalInput").ap()
    maskst = nc.dram_tensor("maskst", [4, P, NCHUNK], BF16, kind="ExternalInput").ap()
    onesc = nc.dram_tensor("onesc", [P, 1], F32R, kind="ExternalInput").ap()
    onesr = nc.dram_tensor("onesr", [1, P], F32R, kind="ExternalInput").ap()
    out = nc.dram_tensor("out", [S, D], F32, kind="ExternalOutput").ap()

    xT_v = xT.rearrange("(t p) s -> p t s", p=P)          # [128, 16, 2048]
    wq_v = wq.rearrange("(t p) o -> p t o", p=P)          # [128, 16, 512]
    wk_v = wk.rearrange("(t p) o -> p t o", p=P)          # [128, 16, 128]
    wv_v = wv.rearrange("(t p) o -> p t o", p=P)
    wo_v = wo.rearrange("(h p) d -> p h d", p=P)          # [128, 4, 2048]
    mask_v = maskst.rearrange("j p c -> p j c")           # [128, 4, 512]
    out_v = out.rearrange("(t p) d -> t p d", p=P)        # [16, 128, 2048]

    with tile.TileContext(nc) as tc:
        with ExitStack() as ctx:
            pers = ctx.enter_context(tc.tile_pool(name="pers", bufs=1))
            psum = ctx.enter_context(tc.tile_pool(name="psum", bufs=8, space="PSUM"))
            xpool = ctx.enter_context(tc.tile_pool(name="xpool", bufs=6))
            spool = ctx.enter_context(tc.tile_pool(name="spool", bufs=2))
            epool = ctx.enter_context(tc.tile_pool(name="epool", bufs=4))
            wopool = ctx.enter_context(tc.tile_pool(name="wopool", bufs=3))
            opool = ctx.enter_context(tc.tile_pool(name="opool", bufs=2))
            cpool = ctx.enter_context(tc.tile_pool(name="cpool", bufs=2))

            _bank_n = [0]

            def bank():
                _bank_n[0] += 1
                return psum.tile([P, NCHUNK], F32, tag="bank",
                                 name=f"bank{_bank_n[0]}")

            # ---- persistent tiles ----
            wq_t = pers.tile([P, DT, GD], F32R, tag="wq")
            wk_t = pers.tile([P, DT, HD], F32R, tag="wk")
            wv_t = pers.tile([P, DT, HD], F32R, tag="wv")
            cos_t = pers.tile([P, S], F32, tag="cos")
            sin_t = pers.tile([P, S], F32, tag="sin")
            prot_t = pers.tile([P, HD], F32R, tag="prot")
            ident = pers.tile([P, P], F32, tag="ident")
            mask_t = pers.tile([P, 4, NCHUNK], BF16, tag="mask")
            qf = pers.tile([P, HPG, S], F32R, tag="qf")       # roped Q^T, 4 heads
            kf = pers.tile([P, S], F32R, tag="kf")            # roped K^T
            vnat = pers.tile([P, ST, HD], F32R, tag="vnat")   # V natural [s, hd]
            ones_col = pers.tile([P, 1], F32R, tag="ones_col")    # [K=128, M=1]
            ones_row = pers.tile([1, P], F32R, tag="ones_row")    # [K=1, M=128]

            # Per-dt weight loads interleaved with the first s-chunk's x tiles
            # so the first matmuls start after ~1 MB of DMA, not ~25 MB.
            xt0 = []
            for dt in range(DT):
                nc.sync.dma_start(wq_t[:, dt, :], wq_v[:, dt, :])
                nc.sync.dma_start(wk_t[:, dt, :], wk_v[:, dt, :])
                nc.sync.dma_start(wv_t[:, dt, :], wv_v[:, dt, :])
                xt = xpool.tile([P, NCHUNK], F32R, tag="xt", name=f"xt0_{dt}")
                nc.sync.dma_start(xt[:], xT_v[:, dt, 0:NCHUNK])
                xt0.append(xt)
            nc.sync.dma_start(cos_t[:], cosT[:])
            nc.sync.dma_start(sin_t[:], sinT[:])
            nc.sync.dma_start(prot_t[:], prot[:])
            nc.sync.dma_start(mask_t[:], mask_v)
            nc.sync.dma_start(ones_col[:], onesc[:])
            nc.sync.dma_start(ones_row[:], onesr[:])
            make_identity(nc, ident[:])

            # ================= Phase A: QKV projection + RoPE + V^T -> V ====
            def rope(dst, src_sb, sc):
                """dst[128,512] (f32r slice) = rope(src_sb [128,512] f32r) for s-chunk sc."""
                cs = cos_t[:, sc * NCHUNK:(sc + 1) * NCHUNK]
                sn = sin_t[:, sc * NCHUNK:(sc + 1) * NCHUNK]
                rotps = bank()
                nc.tensor.matmul(rotps[:], prot_t[:], src_sb, start=True, stop=True)
                t1 = spool.tile([P, NCHUNK], F32, tag="t1")
                nc.vector.tensor_mul(t1[:], rotps[:], sn)
                nc.vector.tensor_mul(dst, src_sb.bitcast(F32), cs)
                nc.vector.tensor_add(dst, dst.bitcast(F32), t1[:])

            for sc in range(SC):
                s0 = sc * NCHUNK
                qps = [bank() for _ in range(HPG)]
                kps = bank()
                vps = bank()
                for dt in range(DT):
                    if sc == 0:
                        xt = xt0[dt]
                    else:
                        xt = xpool.tile([P, NCHUNK], F32R, tag="xt")
                        nc.sync.dma_start(xt[:], xT_v[:, dt, s0:s0 + NCHUNK])
                    st_flag = dt == 0
                    sp_flag = dt == DT - 1
                    for h in range(HPG):
                        nc.tensor.matmul(
                            qps[h][:], wq_t[:, dt, h * HD:(h + 1) * HD], xt[:],
                            start=st_flag, stop=sp_flag)
                    nc.tensor.matmul(kps[:], wk_t[:, dt, :], xt[:],
                                     start=st_flag, stop=sp_flag)
                    nc.tensor.matmul(vps[:], wv_t[:, dt, :], xt[:],
                                     start=st_flag, stop=sp_flag)
                # Q rope
                for h in range(HPG):
                    qsb = spool.tile([P, NCHUNK], F32R, tag="qsb")
                    nc.scalar.copy(qsb[:], qps[h][:])
                    rope(qf[:, h, s0:s0 + NCHUNK], qsb[:], sc)
                # K rope
                ksb = spool.tile([P, NCHUNK], F32R, tag="qsb")
                nc.scalar.copy(ksb[:], kps[:])
                rope(kf[:, s0:s0 + NCHUNK], ksb[:], sc)
                # V: evict V^T chunk, transpose into natural [s, hd] tiles
                vsb = spool.tile([P, NCHUNK], F32, tag="vsb")
                nc.scalar.copy(vsb[:], vps[:])
                for j in range(4):
                    tps = bank()
                    nc.tensor.transpose(
                        tps[:, :P], vsb[:, j * P:(j + 1) * P], ident[:])
                    nc.any.tensor_copy(vnat[:, sc * 4 + j, :], tps[:, :P])

            # ============ Phase B/C: attention per (q-chunk, head) + out-proj
            def emit_norm(ctxq, h, ctxps, denps):
                # normalize: ctx * (1/den) broadcast across partitions.
                # Deferred one head so the reciprocal round trip hides
                # behind the next head's matmul stream.
                rec32 = spool.tile([1, NCHUNK], F32, tag="rec32")
                nc.vector.reciprocal_approx_fast(rec32[:], denps[0:1, :])
                rec = spool.tile([1, NCHUNK], F32R, tag="rec")
                nc.vector.tensor_copy(rec[:], rec32[:])
                bps = bank()
                nc.tensor.matmul(bps[:], ones_row[:], rec[:],
                                 start=True, stop=True)
                bsb = spool.tile([P, NCHUNK], F32, tag="bsb")
                nc.scalar.copy(bsb[:], bps[:])
                nc.vector.tensor_mul(ctxq[:, h, :], ctxps[:], bsb[:])

            LOOKAHEAD = 2
            for qc in range(SC):
                q0 = qc * NCHUNK
                nki = 4 * qc + 4
                ctxq = cpool.tile([P, HPG, NCHUNK], F32R, tag="ctxq")
                pending = None
                for h in range(HPG):
                    ctxps = bank()
                    denps = bank()
                    ets = {}

                    def do_scores(ki):
                        sps = bank()
                        nc.tensor.matmul(
                            sps[:], kf[:, ki * P:(ki + 1) * P],
                            qf[:, h, q0:q0 + NCHUNK], start=True, stop=True)
                        et = epool.tile([P, NCHUNK], F32R, tag="et",
                                        name=f"et{qc}_{h}_{ki}")
                        nc.scalar.activation(et[:], sps[:], AF.Exp, scale=SCALE)
                        if ki >= 4 * qc:
                            nc.vector.tensor_mul(
                                et[:], et[:].bitcast(F32),
                                mask_t[:, ki - 4 * qc, :])
                        ets[ki] = et

                    def do_ctx(ki):
                        et = ets.pop(ki)
                        nc.tensor.matmul(ctxps[:], vnat[:, ki, :], et[:],
                                         start=(ki == 0), stop=(ki == nki - 1))
                        nc.tensor.matmul(denps[0:1, :], ones_col[:], et[:],
                                         start=(ki == 0), stop=(ki == nki - 1))

                    for ki in range(nki):
                        do_scores(ki)
                        if ki >= LOOKAHEAD:
                            do_ctx(ki - LOOKAHEAD)
                    for ki in range(max(0, nki - LOOKAHEAD), nki):
                        do_ctx(ki)
                    if pending is not None:
                        emit_norm(ctxq, *pending)
                    pending = (h, ctxps, denps)
                emit_norm(ctxq, *pending)
                # out-projection for this q-chunk (s-tiles qc*4..qc*4+3)
                for dc in range(SC):
                    wot = wopool.tile([P, HPG, NCHUNK], F32R, tag="wot")
                    nc.sync.dma_start(
                        wot[:], wo_v[:, :, dc * NCHUNK:(dc + 1) * NCHUNK])
                    for st in range(4):
                        stq = qc * 4 + st
                        ops = bank()
                        for h in range(HPG):
                            nc.tensor.matmul(
                                ops[:], ctxq[:, h, st * P:(st + 1) * P],
                                wot[:, h, :],
                                start=(h == 0), stop=(h == HPG - 1))
                        osb = opool.tile([P, NCHUNK], F32, tag="osb")
                        nc.any.tensor_copy(osb[:], ops[:])
                        nc.sync.dma_start(
                            out_v[stq, :, dc * NCHUNK:(dc + 1) * NCHUNK], osb[:])

    nc.compile()
    return nc


def _host_consts():
    i = np.arange(0, HD, 2, dtype=np.float32)
    inv = (1.0 / (10000.0 ** (i / HD))).astype(np.float32)      # [64]
    t = np.arange(S, dtype=np.float32)
    freqs = t[:, None] * inv[None, :]                           # [S, 64] f32
    emb = np.concatenate([freqs, freqs], axis=1)                # [S, 128]
    cosT = np.cos(emb).T.astype(np.float32).copy()              # [128, S]
    sinT = np.sin(emb).T.astype(np.float32).copy()
    prot = np.zeros((HD, HD), dtype=np.float32)
    half = HD // 2
    for ii in range(half):
        prot[ii + half, ii] = -1.0     # rot[i] = -x[i+64], i < 64
    for ii in range(half, HD):
        prot[ii - half, ii] = 1.0      # rot[i] =  x[i-64], i >= 64
    # diagonal causal masks (transposed layout): block j keeps k<=q,
    # mask[j, r, c] = 1 if 128*j + r <= c else 0
    r = np.arange(P)[:, None]
    c = np.arange(NCHUNK)[None, :]
    import ml_dtypes
    maskst = np.stack(
        [(P * j + r <= c).astype(ml_dtypes.bfloat16) for j in range(4)], axis=0)
    return cosT, sinT, prot, maskst


def _in_maps(x, Wq, Wk, Wv, Wo):
    cosT, sinT, prot, maskst = _host_consts()
    maps = []
    for c in range(8):
        b, g = c // 4, c % 4
        maps.append({
            "xT": np.ascontiguousarray(x[b].T),
            "wq": np.ascontiguousarray(Wq[:, g * GD:(g + 1) * GD]),
            "wk": np.ascontiguousarray(Wk[:, g * HD:(g + 1) * HD]),
            "wv": np.ascontiguousarray(Wv[:, g * HD:(g + 1) * HD]),
            "wo": np.ascontiguousarray(Wo[g * GD:(g + 1) * GD, :]),
            "cosT": cosT, "sinT": sinT, "prot": prot, "maskst": maskst,
            "onesc": np.ones((P, 1), dtype=np.float32),
            "onesr": np.ones((1, P), dtype=np.float32),
        })
    return maps


def run(x, Wq, Wk, Wv, Wo, trace=False, **trace_kw):
    if "nc" not in _CACHE:
        _CACHE["nc"] = _build()
    nc = _CACHE["nc"]
    maps = _in_maps(
        np.asarray(x, dtype=np.float32), np.asarray(Wq, dtype=np.float32),
        np.asarray(Wk, dtype=np.float32), np.asarray(Wv, dtype=np.float32),
        np.asarray(Wo, dtype=np.float32))
    res = run_bass_kernel_spmd(
        nc, maps, core_ids=list(range(8)), trace=trace, **trace_kw)
    parts = [res.results[c]["out"] for c in range(8)]
    full = np.stack([
        parts[0] + parts[1] + parts[2] + parts[3],
        parts[4] + parts[5] + parts[6] + parts[7],
    ]).astype(np.float32)
    return full, res


def kernel(x, Wq, Wk, Wv, Wo, mask=None):
    full, _ = run(x, Wq, Wk, Wv, Wo, trace=False)
    return full
